# revision 24
# baseline (speedup 1.0000x reference)
"""GCN (8-layer, 16 GCNConv) on 8 TRN2 NeuronCores.

Strategy (v2):
- dst-partition nodes across 8 cores (6250 each); weights replicated.
- norm separability: norm[e] = dis[src]*dis[dst], so each conv is
    g = dis * (h @ W)         (node-major, per-core slice)
    AllGather g (split into two half-collectives a/b)
    agg[f,d] = sum_e g_fm[src[e]] onehot[e,d]   via PE matmuls over
               128-edge chunks (msgs gathered edge-major by SWDGE dma_gather)
    h' = relu?(dis * agg + b)  (feature-major)
- self-loop folded into the stream-a PSUM accumulation as one
  transpose-matmul per 128-dst block (start=True), so each block needs
  only one DVE drain per stream (copy for a, add for b) — no PE<->DVE
  ping-pong inside the scatter loop.
- one-hot tiles: ids are broadcast-expanded on the Scalar (ACT) engine,
  then a flat 2D is_equal on DVE (mode knob GCN_OH_MODE).
- gathers: SEG=2048-slot segments, 4 SWDGE queues, deep msg pool, and
  segments are emitted several blocks ahead (GCN_PF) so SWDGE streams
  without stalling on consumers.
- edges host-sorted by (stream, dst block); per-block chunk counts shared
  across cores (max), pad slots gather row 0 with onehot id -1.
- int16 gather indices: the a/b split keeps indices < 25600.
- final mean-pool via matmul with host-built pooling matrix + AllReduce.
"""
import numpy as np
import concourse.bass as bass
import concourse.mybir as mybir
import concourse.bacc as bacc
import concourse.tile as tile
from concourse.bass_utils import run_bass_kernel_spmd

import os
N = 50000
E = 600000
D = 128
L = 8
NCONV = int(os.environ.get("GCN_NCONV", 2 * L))
SKIP_COLL = os.environ.get("GCN_SKIP_COLL", "") == "1"
SKIP_GDMA = os.environ.get("GCN_SKIP_GDMA", "") == "1"
SKIP_STREAMS = os.environ.get("GCN_SKIP_STREAMS", "") == "1"
SKIP_GATHER = os.environ.get("GCN_SKIP_GATHER", "") == "1"
DUMP_H = os.environ.get("GCN_DUMP_H", "") == "1"
NOSELF = os.environ.get("GCN_NOSELF", "") == "1"
FAKE_DRAIN = os.environ.get("GCN_FAKE_DRAIN", "") == "1"
NOEPI = os.environ.get("GCN_NOEPI", "") == "1"
BLIND_GATHER = os.environ.get("GCN_BLIND_GATHER", "") == "1"
PREGATHER = os.environ.get("GCN_PREGATHER", "") == "1"
C = 8
NPC = N // C              # 6250 nodes per core
NB = (NPC + 127) // 128   # 49 blocks
NPAD = NB * 128           # 6272
CH_A = 25                 # chunks 0..24 -> stream a
HALF_A = CH_A * 128       # 3200 nodes (a-half, incl none padded)
HALF_B = NPAD - HALF_A    # 3072 node slots (b-half, incl 22 pads)
NH_A = HALF_A * C         # 25600 rows in g_full_a
NH_B = HALF_B * C         # 24576 rows in g_full_b
SEG = int(os.environ.get("GCN_SEG", 4096))   # slots per dma_gather call
SEGC = SEG // 128         # chunks per segment
NG = 64                   # graphs
NQ = int(os.environ.get("GCN_NQ", 4))
MSGBUF = int(os.environ.get("GCN_NBUF", 4))
OHBUF = int(os.environ.get("GCN_OHBUF", 3))
IEBUF = int(os.environ.get("GCN_IEBUF", 2))
PF = int(os.environ.get("GCN_PF", 6))        # prefetch lookahead in blocks
OH_MODE = int(os.environ.get("GCN_OH_MODE", 1))  # 0=DVE bcast, 1=ACT+DVE, 2=alt, 3=DMA

f32 = mybir.dt.float32
bf16 = mybir.dt.bfloat16
i16 = mybir.dt.int16
i32 = mybir.dt.int32
AT = mybir.AluOpType
ACTF = mybir.ActivationFunctionType


def _wrap16(vals: np.ndarray) -> np.ndarray:
    """slot i -> [i % 16, i // 16], replicated to 128 partitions."""
    n = len(vals)
    base = vals.astype(np.int16).reshape(n // 16, 16).T   # [16, n//16]
    return np.ascontiguousarray(np.tile(base, (8, 1)))


def host_prep(x, edge_index, batch):
    src = edge_index[0].astype(np.int64)
    dst = edge_index[1].astype(np.int64)
    deg = np.bincount(dst, minlength=N).astype(np.float64) + 1.0
    dis = (1.0 / np.sqrt(deg)).astype(np.float32)

    r = src // NPC
    k = src % NPC
    kb = k // 128
    kp = k % 128
    stream = (kb >= CH_A).astype(np.int64)              # 0 = a, 1 = b
    # chunk-major rows: a: r*3200 + p*25 + b ; b: r*3072 + p*24 + (b-25)
    loc = np.where(stream == 0,
                   r * HALF_A + kp * CH_A + kb,
                   r * HALF_B + kp * (NB - CH_A) + (kb - CH_A)).astype(np.int64)
    core_of = dst // NPC
    dloc = dst % NPC
    blk = dloc // 128
    id_in_blk = dloc % 128

    key = (core_of * 2 + stream) * NB + blk
    order = np.argsort(key, kind="stable")
    s_loc = loc[order]
    s_id = id_in_blk[order]
    gcnt = np.bincount(key, minlength=C * 2 * NB).reshape(C, 2, NB)
    goff = np.zeros(C * 2 * NB + 1, np.int64)
    np.cumsum(gcnt.reshape(-1), out=goff[1:])

    # shared chunks-per-block (max over cores), per stream
    CPB = [np.maximum.reduce(-(-gcnt[:, s, :] // 128), axis=0) for s in range(2)]
    chunk_blocks = [np.repeat(np.arange(NB), CPB[s]) for s in range(2)]
    S = [int(CPB[s].sum()) * 128 for s in range(2)]
    blk_chunk_start = [np.concatenate([[0], np.cumsum(CPB[s])]) for s in range(2)]

    idx_host = [[None] * C for _ in range(2)]
    ids_host = [[None] * C for _ in range(2)]
    oh_host = [[None] * C for _ in range(2)]
    for s in range(2):
        for c in range(C):
            ia = np.zeros(S[s], np.int64)
            da = np.full(S[s], -1.0, np.float32)
            for b in range(NB):
                g = (c * 2 + s) * NB + b
                cnt = goff[g + 1] - goff[g]
                o = int(blk_chunk_start[s][b]) * 128
                ia[o:o + cnt] = s_loc[goff[g]:goff[g + 1]]
                da[o:o + cnt] = s_id[goff[g]:goff[g + 1]]
            idx_host[s][c] = _wrap16(ia)
            import ml_dtypes
            ids = np.ascontiguousarray(
                da.reshape(S[s] // 128, 128).T)          # [128, S/128]
            ids_host[s][c] = ids.astype(ml_dtypes.bfloat16)
            # pre-expanded one-hot: [128, S] with oh[p, ch*128+d] =
            # (ids[p, ch] == d); pads (-1) match nothing
            oh = np.equal(ids[:, :, None],
                          np.arange(128, dtype=np.float32)[None, None, :])
            oh_host[s][c] = np.ascontiguousarray(
                oh.reshape(128, S[s]).astype(ml_dtypes.bfloat16))

    # segment calls: (slot_off, nslots, chunk0, nchunks)
    calls = []
    for s in range(2):
        cl = []
        off = 0
        while off < S[s]:
            n = min(SEG, S[s] - off)
            cl.append((off, n, off // 128, n // 128))
            off += n
        calls.append(cl)

    # per-core aux arrays
    dis_pad = np.zeros((C, NPAD), np.float32)
    for c in range(C):
        dis_pad[c, :NPC] = dis[c * NPC:(c + 1) * NPC]
    dis_nm = dis_pad.reshape(C, NB, 128).transpose(0, 2, 1).copy()   # [C,128,NB]
    dis_fm = np.repeat(dis_pad[:, None, :], 128, axis=1)             # [C,128,NPAD]

    x_fm = np.zeros((C, 128, NPAD), np.float32)
    for c in range(C):
        x_fm[c, :, :NPC] = x[c * NPC:(c + 1) * NPC].T

    cnt = np.bincount(batch.astype(np.int64), minlength=NG).astype(np.float64)
    w = (1.0 / np.maximum(cnt, 1.0)).astype(np.float32)
    pool_mat = np.zeros((C, NPAD, NG), np.float32)
    for c in range(C):
        bl = batch[c * NPC:(c + 1) * NPC].astype(np.int64)
        pool_mat[c, np.arange(NPC), bl] = w[bl]
    pool_t = pool_mat.reshape(C, NB, 128, NG).transpose(0, 2, 1, 3).reshape(
        C, 128, NB * NG).copy()

    return dict(dis_nm=dis_nm, dis_fm=dis_fm, x_fm=x_fm, pool_t=pool_t,
                idx_host=idx_host, ids_host=ids_host, oh_host=oh_host,
                calls=calls, CPB=CPB, chunk_blocks=chunk_blocks, S=S,
                blk_chunk_start=blk_chunk_start)


def build_program(nc, st):
    calls = st["calls"]
    CPB = st["CPB"]
    blk_start = st["blk_chunk_start"]
    S = st["S"]

    # ---- I/O ----
    x_in = nc.dram_tensor("x_fm", [128, NPAD], f32, kind="ExternalInput")
    w_in = nc.dram_tensor("wcat", [128, NCONV * 128], f32, kind="ExternalInput")
    b_in = nc.dram_tensor("bcat", [128, NCONV], f32, kind="ExternalInput")
    dnm_in = nc.dram_tensor("dis_nm", [128, NB], f32, kind="ExternalInput")
    dfm_in = nc.dram_tensor("dis_fm", [128, NPAD], f32, kind="ExternalInput")
    pool_in = nc.dram_tensor("pool_t", [128, NB * NG], f32, kind="ExternalInput")
    ident_in = nc.dram_tensor("ident", [128, 128], f32, kind="ExternalInput")
    idx_in = [nc.dram_tensor(f"idx{s}", [128, S[s] // 16], i16,
                             kind="ExternalInput") for s in range(2)]
    ids_in = [nc.dram_tensor(f"ids{s}", [128, S[s] // 128], bf16,
                             kind="ExternalInput") for s in range(2)]
    oh_in = [nc.dram_tensor(f"ohd{s}", [128, S[s]], bf16,
                            kind="ExternalInput") for s in range(2)]
    out_t = nc.dram_tensor("out", [NG, 128], f32, kind="ExternalOutput")
    hdump_t = nc.dram_tensor("hdump", [128, NPAD], f32,
                             kind="ExternalOutput") if DUMP_H else None

    g_slice = [nc.dram_tensor("g_slice0", [HALF_A, 128], bf16, kind="Internal"),
               nc.dram_tensor("g_slice1", [HALF_B, 128], bf16, kind="Internal")]
    g_full = [nc.dram_tensor("g_full0", [NH_A, 128], bf16, kind="Internal",
                             addr_space="Shared"),
              nc.dram_tensor("g_full1", [NH_B, 128], bf16, kind="Internal",
                             addr_space="Shared")]
    ar_in = nc.dram_tensor("ar_in", [NG, 128], f32, kind="Internal")
    ar_out = nc.dram_tensor("ar_out", [NG, 128], f32, kind="Internal",
                            addr_space="Shared")
    rg = [list(range(C))]

    def ap3(t, off_elems, dims):
        return bass.AP(t, off_elems, dims)

    with tile.TileContext(nc) as tc:
        with tc.tile_pool(name="const", bufs=1) as cp, \
             tc.tile_pool(name="state", bufs=1) as sp, \
             tc.tile_pool(name="ph", bufs=2, space="PSUM") as php, \
             tc.tile_pool(name="pagg", bufs=int(os.environ.get("GCN_PAGG", 5)), space="PSUM") as pap:

            b_t = cp.tile([128, NCONV], f32, tag="b")
            dnm_t = cp.tile([128, NB], f32, tag="dnm")
            dfm_t = cp.tile([128, NPAD], f32, tag="dfm")
            ident_t = cp.tile([128, 128], f32, tag="ident")
            ident_bf = cp.tile([128, 128], bf16, tag="identbf")
            iota_seg_f = cp.tile([128, SEG], bf16, tag="iosegf")

            idx_res = [cp.tile([128, S[s] // 16], i16, tag=f"idxr{s}",
                               name=f"idxr{s}") for s in range(2)]
            ids_res = [cp.tile([128, S[s] // 128], bf16, tag=f"idsr{s}",
                               name=f"idsr{s}") for s in range(2)]
            h0 = sp.tile([128, NPAD], f32, tag="h0")
            h1 = sp.tile([128, NPAD], f32, tag="h1")
            g_nm = sp.tile([128, NPAD], bf16, tag="gnm")
            hb = [h0, h1]
            scr = (sp.tile([128, 128], f32, tag="scr", name="scr")
                   if FAKE_DRAIN else None)
            if FAKE_DRAIN:
                nc.vector.memset(scr[:], 0.0)
                nc.vector.memset(h1[:], 0.0)

            nc.sync.dma_start(b_t[:], b_in[:])
            nc.sync.dma_start(dnm_t[:], dnm_in[:])
            nc.sync.dma_start(dfm_t[:], dfm_in[:])
            nc.sync.dma_start(ident_t[:], ident_in[:])
            nc.vector.tensor_copy(ident_bf[:], ident_t[:])
            nc.sync.dma_start(h0[:], x_in[:])
            for s in range(2):
                nc.sync.dma_start(idx_res[s][:], idx_in[s][:])
                nc.sync.dma_start(ids_res[s][:], ids_in[s][:])
            nc.gpsimd.iota(iota_seg_f[:], pattern=[[0, SEGC], [1, 128]],
                           base=0, channel_multiplier=0,
                           allow_small_or_imprecise_dtypes=True)

            def bs(b):
                return slice(b * 128, (b + 1) * 128)

            def emit_g_dmas(half):
                """One contiguous DMA: g_nm cols -> chunk-major slice rows."""
                if half == 0:
                    nc.sync.dma_start(
                        ap3(g_slice[0], 0, [[HALF_A, 128], [1, HALF_A]]),
                        g_nm[:, 0:HALF_A])
                else:
                    nc.sync.dma_start(
                        ap3(g_slice[1], 0, [[HALF_B, 128], [1, HALF_B]]),
                        g_nm[:, HALF_A:NPAD])

            mp = tc.alloc_tile_pool(name="msg", bufs=MSGBUF)
            op = tc.alloc_tile_pool(name="oh", bufs=OHBUF)
            iep = tc.alloc_tile_pool(name="ie", bufs=IEBUF)
            tp = tc.alloc_tile_pool(name="meta", bufs=3)
            blp = (tc.alloc_tile_pool(name="blind", bufs=int(os.environ.get("GCN_BLBUF", 4)))
                   if (BLIND_GATHER or PREGATHER) else None)

            qctr = [0]
            PREP = os.environ.get("GCN_PREP", "") == "1"
            dma_sems = ([nc.alloc_semaphore(f"swdge_dma{q}") for q in range(NQ)]
                        if PREP else None)
            if PREGATHER:
                # all gathers bunched up-front (timing experiment): same
                # total SWDGE volume as 16 convs' worth, no interleaving
                for _cv in range(NCONV):
                    for s in range(2):
                        for si in range(len(calls[s])):
                            off, n, c0, nch = calls[s][si]
                            bt = blp.tile([128, SEGC, 128], bf16, tag="blind",
                                          name="bt")
                            q = qctr[0] % NQ
                            if PREP:
                                nc.gpsimd.dma_gather(
                                    bt[:, :nch, :], g_full[s][:],
                                    idx_res[s][:, off // 16:(off + n) // 16],
                                    num_idxs=n, num_idxs_reg=n, elem_size=128,
                                    single_packet=False, prepare_only=True,
                                    sem=dma_sems[q], queue_num=q)
                                nc.gpsimd.trigger_dma(count=None, queue_num=q)
                            else:
                                nc.gpsimd.dma_gather(
                                    bt[:, :nch, :], g_full[s][:],
                                    idx_res[s][:, off // 16:(off + n) // 16],
                                    num_idxs=n, num_idxs_reg=n, elem_size=128,
                                    single_packet=False,
                                    queue_num=q)
                            qctr[0] += 1

            for cv in range(NCONV):
                h_cur = hb[cv % 2]
                h_nxt = hb[(cv + 1) % 2]
                w_t = tp.tile([128, 128], f32, tag="wt", bufs=2)
                nc.sync.dma_start(w_t[:], w_in[:, cv * 128:(cv + 1) * 128])
                wsl = w_t[:]

                # --- phase 1: g = dis * (h @ W), node-major; fire halves ---
                for b in range(NB):
                    ph = php.tile([128, 128], f32, tag="ph")
                    nc.tensor.matmul(ph[:], h_cur[:, bs(b)], wsl,
                                     start=True, stop=True)
                    nc.scalar.activation(g_nm[:, bs(b)], ph[:], ACTF.Copy,
                                         scale=dnm_t[:, b:b + 1])
                    if b == 24:
                        if not SKIP_GDMA:
                            emit_g_dmas(0)
                        if not SKIP_COLL:
                            nc.gpsimd.collective_compute(
                                "AllGather", AT.bypass, replica_groups=rg,
                                ins=[g_slice[0][:]], outs=[g_full[0][:]])
                if not SKIP_GDMA:
                    emit_g_dmas(1)
                # NOTE: AllGather-b is emitted after pass-A's gathers are
                # enqueued so it does not block their descriptor-gen on the
                # in-order Pool queue (the collective sits on gpsimd).

                # --- phase 2: scatter-add, stream-major, selfloop folded ---
                emitted = [-1, -1]
                msg_t = [{}, {}]
                oh_t = [{}, {}]

                def emit_seg(s):
                    si = emitted[s] + 1
                    off, n, c0, nch = calls[s][si]
                    msg = mp.tile([128, SEGC, 128], bf16, tag="msg")
                    if BLIND_GATHER:
                        bt = blp.tile([128, SEGC, 128], bf16, tag="blind",
                                      name="bt")
                        nc.gpsimd.dma_gather(
                            bt[:, :nch, :], g_full[s][:],
                            idx_res[s][:, off // 16:(off + n) // 16],
                            num_idxs=n, num_idxs_reg=n, elem_size=128,
                            single_packet=False,
                            queue_num=qctr[0] % NQ)
                        qctr[0] += 1
                        nc.vector.memset(msg[:, :nch, :], 0.0)
                    elif not (SKIP_GATHER or PREGATHER):
                        nc.gpsimd.dma_gather(
                            msg[:, :nch, :], g_full[s][:],
                            idx_res[s][:, off // 16:(off + n) // 16],
                            num_idxs=n, num_idxs_reg=n, elem_size=128,
                            single_packet=False,
                            queue_num=qctr[0] % NQ)
                        qctr[0] += 1
                    else:
                        nc.vector.memset(msg[:, :nch, :], 0.0)
                    oh = op.tile([128, SEG], bf16, tag="oh")
                    sl = ids_res[s][:, c0:c0 + nch]
                    in1 = bass.AP(sl.tensor, sl.offset, sl.ap + [[0, 128]])
                    mode = OH_MODE if OH_MODE != 2 else (si % 2)
                    if mode == 3:
                        # one-hots precomputed on host, streamed via HWDGE
                        nc.sync.dma_start(
                            oh[:, :nch * 128],
                            oh_in[s][:, c0 * 128:(c0 + nch) * 128])
                    elif mode == 1:
                        ie = iep.tile([128, SEG], bf16, tag="ie")
                        nc.scalar.activation(
                            ie[:].rearrange("p (c d) -> p c d", d=128)[:, :nch, :],
                            in1, ACTF.Copy)
                        nc.vector.tensor_tensor(
                            oh[:, :nch * 128], iota_seg_f[:, :nch * 128],
                            ie[:, :nch * 128], AT.is_equal)
                    else:
                        nc.vector.tensor_tensor(
                            oh[:].rearrange("p (c d) -> p c d", d=128)[:, :nch, :],
                            iota_seg_f[:].rearrange("p (c d) -> p c d", d=128)[:, :nch, :],
                            in1, AT.is_equal)
                    msg_t[s][si] = msg
                    oh_t[s][si] = oh
                    emitted[s] = si

                def ensure(s, blk):
                    bp = min(blk, NB - 1)
                    tch = int(blk_start[s][bp]) + max(int(CPB[s][bp]), 1) - 1
                    tsi = min(tch // SEGC, len(calls[s]) - 1)
                    while emitted[s] < tsi:
                        emit_seg(s)

                for s in range(2):
                    if s == 1 and not SKIP_COLL:
                        nc.gpsimd.collective_compute(
                            "AllGather", AT.bypass, replica_groups=rg,
                            ins=[g_slice[1][:]], outs=[g_full[1][:]])
                    for b in range(NB):
                        nch_b = 0 if SKIP_STREAMS else int(CPB[s][b])
                        if s == 1 and nch_b == 0:
                            continue
                        if not SKIP_STREAMS:
                            ensure(s, b + PF)
                        pa = pap.tile([128, 128], f32, tag="pagg")
                        started = False
                        if s == 0 and not NOSELF:
                            nc.tensor.matmul(pa[:], g_nm[:, bs(b)], ident_bf[:],
                                             start=True, stop=(nch_b == 0))
                            started = True
                        c_lo = int(blk_start[s][b])
                        for j in range(nch_b):
                            ch = c_lo + j
                            si = ch // SEGC
                            jj = ch % SEGC
                            nc.tensor.matmul(
                                pa[:], msg_t[s][si][:, jj, :],
                                oh_t[s][si][:, jj * 128:(jj + 1) * 128],
                                start=(not started),
                                stop=(j == nch_b - 1))
                            started = True
                        if not started:
                            continue
                        dtgt = scr[:, 0:128] if FAKE_DRAIN else h_nxt[:, bs(b)]
                        if s == 0:
                            nc.vector.tensor_copy(dtgt, pa[:])
                        else:
                            nc.vector.tensor_tensor(dtgt, pa[:], dtgt, AT.add)

                # --- epilogue: h' = relu?(dis * agg + bias), quarter-row
                # pieces so next conv's h@W can start early ---
                QN = NPAD // 4
                for q in range(4 if not NOEPI else 0):
                    qs = slice(q * QN, (q + 1) * QN)
                    nc.vector.tensor_tensor(h_nxt[:, qs], h_nxt[:, qs],
                                            dfm_t[:, qs], AT.mult)
                    if cv % 2 == 0:
                        nc.scalar.activation(h_nxt[:, qs], h_nxt[:, qs],
                                             ACTF.Relu,
                                             bias=b_t[:, cv:cv + 1], scale=1.0)
                    else:
                        nc.vector.tensor_scalar(h_nxt[:, qs], h_nxt[:, qs],
                                                b_t[:, cv:cv + 1], None, AT.add)

            for p in ((blp,) if (BLIND_GATHER or PREGATHER) else ()) + (tp, iep, op, mp):
                p.release()

            # ---- mean pool + AllReduce ----
            h_fin = hb[NCONV % 2]
            if DUMP_H:
                nc.sync.dma_start(hdump_t[:], h_fin[:])
            tailp = tc.alloc_tile_pool(name="tail", bufs=1)
            pool_tile = tailp.tile([128, NB * NG], f32, tag="poolm")
            nc.sync.dma_start(pool_tile[:], pool_in[:])
            hnm = tailp.tile([128, NPAD], f32, tag="hnm")
            for b in range(NB):
                pt = php.tile([128, 128], f32, tag="ph")
                nc.tensor.transpose(pt[:], h_fin[:, bs(b)], ident_t[:])
                nc.vector.tensor_copy(hnm[:, bs(b)], pt[:])
            ppool = pap.tile([NG, 128], f32, tag="ppool", bufs=1)
            for b in range(NB):
                nc.tensor.matmul(ppool[:], pool_tile[:, b * NG:(b + 1) * NG],
                                 hnm[:, bs(b)], start=(b == 0),
                                 stop=(b == NB - 1))
            pres = sp.tile([NG, 128], f32, tag="pres")
            nc.vector.tensor_copy(pres[:], ppool[:])
            nc.sync.dma_start(ar_in[:], pres[:])
            if not SKIP_COLL:
                nc.gpsimd.collective_compute(
                    "AllReduce", AT.add, replica_groups=rg,
                    ins=[ar_in[:]], outs=[ar_out[:]])
            ores = sp.tile([NG, 128], f32, tag="ores")
            nc.sync.dma_start(ores[:], ar_out[:] if not SKIP_COLL else ar_in[:])
            nc.sync.dma_start(out_t[:], ores[:])
            tailp.release()
    return nc


def kernel(x, edge_index, batch, W1, b1, W2, b2, _want_trace=False, _want_res=False):
    x = np.asarray(x)
    edge_index = np.asarray(edge_index)
    batch = np.asarray(batch)
    W1, b1, W2, b2 = (np.asarray(a) for a in (W1, b1, W2, b2))

    st = host_prep(x, edge_index, batch)

    wcat = np.zeros((128, 2 * L * 128), np.float32)
    bcat = np.zeros((128, 2 * L), np.float32)
    for l in range(L):
        wcat[:, (2 * l) * 128:(2 * l + 1) * 128] = W1[l]
        wcat[:, (2 * l + 1) * 128:(2 * l + 2) * 128] = W2[l]
        bcat[:, 2 * l] = b1[l]
        bcat[:, 2 * l + 1] = b2[l]
    wcat = np.ascontiguousarray(wcat[:, :NCONV * 128])
    bcat = np.ascontiguousarray(bcat[:, :NCONV])

    nc = bacc.Bacc("TRN2", target_bir_lowering=False, debug=False,
                   enable_asserts=False, num_devices=C,
                   num_swdge_queues=NQ)
    build_program(nc, st)
    nc.compile()

    ident = np.eye(128, dtype=np.float32)
    in_maps = []
    for c in range(C):
        in_maps.append({
            "x_fm": st["x_fm"][c],
            "wcat": wcat, "bcat": bcat,
            "dis_nm": st["dis_nm"][c], "dis_fm": st["dis_fm"][c],
            "pool_t": st["pool_t"][c], "ident": ident,
            "idx0": st["idx_host"][0][c], "idx1": st["idx_host"][1][c],
            "ids0": st["ids_host"][0][c], "ids1": st["ids_host"][1][c],
            "ohd0": st["oh_host"][0][c], "ohd1": st["oh_host"][1][c],
        })

    res = run_bass_kernel_spmd(nc, in_maps, core_ids=list(range(C)),
                               trace=_want_trace)
    out = res.results[0]["out"].astype(np.float32)
    if _want_trace or _want_res:
        return out, res
    return out


# revision 25
# speedup vs baseline: 1.2054x; 1.2054x over previous
"""GCN (8-layer, 16 GCNConv) on 8 TRN2 NeuronCores.

Strategy (v2):
- dst-partition nodes across 8 cores (6250 each); weights replicated.
- norm separability: norm[e] = dis[src]*dis[dst], so each conv is
    g = dis * (h @ W)         (node-major, per-core slice)
    AllGather g (split into two half-collectives a/b)
    agg[f,d] = sum_e g_fm[src[e]] onehot[e,d]   via PE matmuls over
               128-edge chunks (msgs gathered edge-major by SWDGE dma_gather)
    h' = relu?(dis * agg + b)  (feature-major)
- self-loop folded into the stream-a PSUM accumulation as one
  transpose-matmul per 128-dst block (start=True), so each block needs
  only one DVE drain per stream (copy for a, add for b) — no PE<->DVE
  ping-pong inside the scatter loop.
- one-hot tiles: ids are broadcast-expanded on the Scalar (ACT) engine,
  then a flat 2D is_equal on DVE (mode knob GCN_OH_MODE).
- gathers: SEG=2048-slot segments, 4 SWDGE queues, deep msg pool, and
  segments are emitted several blocks ahead (GCN_PF) so SWDGE streams
  without stalling on consumers.
- edges host-sorted by (stream, dst block); per-block chunk counts shared
  across cores (max), pad slots gather row 0 with onehot id -1.
- int16 gather indices: the a/b split keeps indices < 25600.
- final mean-pool via matmul with host-built pooling matrix + AllReduce.
"""
import numpy as np
import concourse.bass as bass
import concourse.mybir as mybir
import concourse.bacc as bacc
import concourse.tile as tile
from concourse.bass_utils import run_bass_kernel_spmd

import os
N = 50000
E = 600000
D = 128
L = 8
NCONV = int(os.environ.get("GCN_NCONV", 2 * L))
SKIP_COLL = os.environ.get("GCN_SKIP_COLL", "") == "1"
SKIP_GDMA = os.environ.get("GCN_SKIP_GDMA", "") == "1"
SKIP_STREAMS = os.environ.get("GCN_SKIP_STREAMS", "") == "1"
SKIP_GATHER = os.environ.get("GCN_SKIP_GATHER", "") == "1"
DUMP_H = os.environ.get("GCN_DUMP_H", "") == "1"
NOSELF = os.environ.get("GCN_NOSELF", "") == "1"
FAKE_DRAIN = os.environ.get("GCN_FAKE_DRAIN", "") == "1"
NOEPI = os.environ.get("GCN_NOEPI", "") == "1"
BLIND_GATHER = os.environ.get("GCN_BLIND_GATHER", "") == "1"
PREGATHER = os.environ.get("GCN_PREGATHER", "") == "1"
C = 8
NPC = N // C              # 6250 nodes per core
NB = (NPC + 127) // 128   # 49 blocks
NPAD = NB * 128           # 6272
CH_A = 25                 # chunks 0..24 -> stream a
HALF_A = CH_A * 128       # 3200 nodes (a-half, incl none padded)
HALF_B = NPAD - HALF_A    # 3072 node slots (b-half, incl 22 pads)
NH_A = HALF_A * C         # 25600 rows in g_full_a
NH_B = HALF_B * C         # 24576 rows in g_full_b
SEG = int(os.environ.get("GCN_SEG", 4096))   # slots per dma_gather call
SEGC = SEG // 128         # chunks per segment
NG = 64                   # graphs
NQ = int(os.environ.get("GCN_NQ", 4))
MSGBUF = int(os.environ.get("GCN_NBUF", 4))
OHBUF = int(os.environ.get("GCN_OHBUF", 3))
IEBUF = int(os.environ.get("GCN_IEBUF", 2))
PF = int(os.environ.get("GCN_PF", 6))        # prefetch lookahead in blocks
OH_MODE = int(os.environ.get("GCN_OH_MODE", 1))  # 0=DVE bcast, 1=ACT+DVE, 2=alt, 3=DMA

f32 = mybir.dt.float32
bf16 = mybir.dt.bfloat16
i16 = mybir.dt.int16
i32 = mybir.dt.int32
AT = mybir.AluOpType
ACTF = mybir.ActivationFunctionType


def _wrap16(vals: np.ndarray) -> np.ndarray:
    """slot i -> [i % 16, i // 16], replicated to 128 partitions."""
    n = len(vals)
    base = vals.astype(np.int16).reshape(n // 16, 16).T   # [16, n//16]
    return np.ascontiguousarray(np.tile(base, (8, 1)))


def host_prep(x, edge_index, batch):
    src = edge_index[0].astype(np.int64)
    dst = edge_index[1].astype(np.int64)
    deg = np.bincount(dst, minlength=N).astype(np.float64) + 1.0
    dis = (1.0 / np.sqrt(deg)).astype(np.float32)

    r = src // NPC
    k = src % NPC
    kb = k // 128
    kp = k % 128
    stream = (kb >= CH_A).astype(np.int64)              # 0 = a, 1 = b
    # chunk-major rows: a: r*3200 + p*25 + b ; b: r*3072 + p*24 + (b-25)
    loc = np.where(stream == 0,
                   r * HALF_A + kp * CH_A + kb,
                   r * HALF_B + kp * (NB - CH_A) + (kb - CH_A)).astype(np.int64)
    core_of = dst // NPC
    dloc = dst % NPC
    blk = dloc // 128
    id_in_blk = dloc % 128

    key = (core_of * 2 + stream) * NB + blk
    order = np.argsort(key, kind="stable")
    s_loc = loc[order]
    s_id = id_in_blk[order]
    gcnt = np.bincount(key, minlength=C * 2 * NB).reshape(C, 2, NB)
    goff = np.zeros(C * 2 * NB + 1, np.int64)
    np.cumsum(gcnt.reshape(-1), out=goff[1:])

    # shared chunks-per-block (max over cores), per stream
    CPB = [np.maximum.reduce(-(-gcnt[:, s, :] // 128), axis=0) for s in range(2)]
    chunk_blocks = [np.repeat(np.arange(NB), CPB[s]) for s in range(2)]
    S = [int(CPB[s].sum()) * 128 for s in range(2)]
    blk_chunk_start = [np.concatenate([[0], np.cumsum(CPB[s])]) for s in range(2)]

    idx_host = [[None] * C for _ in range(2)]
    ids_host = [[None] * C for _ in range(2)]
    oh_host = [[None] * C for _ in range(2)]
    for s in range(2):
        for c in range(C):
            ia = np.zeros(S[s], np.int64)
            da = np.full(S[s], -1.0, np.float32)
            for b in range(NB):
                g = (c * 2 + s) * NB + b
                cnt = goff[g + 1] - goff[g]
                o = int(blk_chunk_start[s][b]) * 128
                ia[o:o + cnt] = s_loc[goff[g]:goff[g + 1]]
                da[o:o + cnt] = s_id[goff[g]:goff[g + 1]]
            idx_host[s][c] = _wrap16(ia)
            import ml_dtypes
            ids = np.ascontiguousarray(
                da.reshape(S[s] // 128, 128).T)          # [128, S/128]
            ids_host[s][c] = ids.astype(ml_dtypes.bfloat16)
            if OH_MODE == 3:
                # pre-expanded one-hot: [128, S] with oh[p, ch*128+d] =
                # (ids[p, ch] == d); pads (-1) match nothing
                oh = np.equal(ids[:, :, None],
                              np.arange(128, dtype=np.float32)[None, None, :])
                oh_host[s][c] = np.ascontiguousarray(
                    oh.reshape(128, S[s]).astype(ml_dtypes.bfloat16))

    # segment calls: (slot_off, nslots, chunk0, nchunks)
    calls = []
    for s in range(2):
        cl = []
        off = 0
        while off < S[s]:
            n = min(SEG, S[s] - off)
            cl.append((off, n, off // 128, n // 128))
            off += n
        calls.append(cl)

    # per-core aux arrays
    dis_pad = np.zeros((C, NPAD), np.float32)
    for c in range(C):
        dis_pad[c, :NPC] = dis[c * NPC:(c + 1) * NPC]
    dis_nm = dis_pad.reshape(C, NB, 128).transpose(0, 2, 1).copy()   # [C,128,NB]
    dis_fm = np.repeat(dis_pad[:, None, :], 128, axis=1)             # [C,128,NPAD]

    x_fm = np.zeros((C, 128, NPAD), np.float32)
    for c in range(C):
        x_fm[c, :, :NPC] = x[c * NPC:(c + 1) * NPC].T

    cnt = np.bincount(batch.astype(np.int64), minlength=NG).astype(np.float64)
    w = (1.0 / np.maximum(cnt, 1.0)).astype(np.float32)
    pool_mat = np.zeros((C, NPAD, NG), np.float32)
    for c in range(C):
        bl = batch[c * NPC:(c + 1) * NPC].astype(np.int64)
        pool_mat[c, np.arange(NPC), bl] = w[bl]
    pool_t = pool_mat.reshape(C, NB, 128, NG).transpose(0, 2, 1, 3).reshape(
        C, 128, NB * NG).copy()

    return dict(dis_nm=dis_nm, dis_fm=dis_fm, x_fm=x_fm, pool_t=pool_t,
                idx_host=idx_host, ids_host=ids_host, oh_host=oh_host,
                calls=calls, CPB=CPB, chunk_blocks=chunk_blocks, S=S,
                blk_chunk_start=blk_chunk_start)


def build_program(nc, st):
    calls = st["calls"]
    CPB = st["CPB"]
    blk_start = st["blk_chunk_start"]
    S = st["S"]

    # ---- I/O ----
    x_in = nc.dram_tensor("x_fm", [128, NPAD], f32, kind="ExternalInput")
    w_in = nc.dram_tensor("wcat", [128, NCONV * 128], f32, kind="ExternalInput")
    b_in = nc.dram_tensor("bcat", [128, NCONV], f32, kind="ExternalInput")
    dnm_in = nc.dram_tensor("dis_nm", [128, NB], f32, kind="ExternalInput")
    dfm_in = nc.dram_tensor("dis_fm", [128, NPAD], f32, kind="ExternalInput")
    pool_in = nc.dram_tensor("pool_t", [128, NB * NG], f32, kind="ExternalInput")
    ident_in = nc.dram_tensor("ident", [128, 128], f32, kind="ExternalInput")
    idx_in = [nc.dram_tensor(f"idx{s}", [128, S[s] // 16], i16,
                             kind="ExternalInput") for s in range(2)]
    ids_in = [nc.dram_tensor(f"ids{s}", [128, S[s] // 128], bf16,
                             kind="ExternalInput") for s in range(2)]
    oh_in = ([nc.dram_tensor(f"ohd{s}", [128, S[s]], bf16,
                             kind="ExternalInput") for s in range(2)]
             if OH_MODE == 3 else None)
    out_t = nc.dram_tensor("out", [NG, 128], f32, kind="ExternalOutput")
    hdump_t = nc.dram_tensor("hdump", [128, NPAD], f32,
                             kind="ExternalOutput") if DUMP_H else None

    g_slice = [nc.dram_tensor("g_slice0", [HALF_A, 128], bf16, kind="Internal"),
               nc.dram_tensor("g_slice1", [HALF_B, 128], bf16, kind="Internal")]
    g_full = [nc.dram_tensor("g_full0", [NH_A, 128], bf16, kind="Internal",
                             addr_space="Shared"),
              nc.dram_tensor("g_full1", [NH_B, 128], bf16, kind="Internal",
                             addr_space="Shared")]
    ar_in = nc.dram_tensor("ar_in", [NG, 128], f32, kind="Internal")
    ar_out = nc.dram_tensor("ar_out", [NG, 128], f32, kind="Internal",
                            addr_space="Shared")
    rg = [list(range(C))]

    def ap3(t, off_elems, dims):
        return bass.AP(t, off_elems, dims)

    with tile.TileContext(nc) as tc:
        with tc.tile_pool(name="const", bufs=1) as cp, \
             tc.tile_pool(name="state", bufs=1) as sp, \
             tc.tile_pool(name="ph", bufs=2, space="PSUM") as php, \
             tc.tile_pool(name="pagg", bufs=int(os.environ.get("GCN_PAGG", 5)), space="PSUM") as pap:

            b_t = cp.tile([128, NCONV], f32, tag="b")
            dnm_t = cp.tile([128, NB], f32, tag="dnm")
            dfm_t = cp.tile([128, NPAD], f32, tag="dfm")
            ident_t = cp.tile([128, 128], f32, tag="ident")
            ident_bf = cp.tile([128, 128], bf16, tag="identbf")
            iota_seg_f = cp.tile([128, SEG], bf16, tag="iosegf")

            idx_res = [cp.tile([128, S[s] // 16], i16, tag=f"idxr{s}",
                               name=f"idxr{s}") for s in range(2)]
            ids_res = [cp.tile([128, S[s] // 128], bf16, tag=f"idsr{s}",
                               name=f"idsr{s}") for s in range(2)]
            h0 = sp.tile([128, NPAD], f32, tag="h0")
            h1 = sp.tile([128, NPAD], f32, tag="h1")
            g_nm = sp.tile([128, NPAD], bf16, tag="gnm")
            hb = [h0, h1]
            scr = (sp.tile([128, 128], f32, tag="scr", name="scr")
                   if FAKE_DRAIN else None)
            if FAKE_DRAIN:
                nc.vector.memset(scr[:], 0.0)
                nc.vector.memset(h1[:], 0.0)

            nc.sync.dma_start(b_t[:], b_in[:])
            nc.sync.dma_start(dnm_t[:], dnm_in[:])
            nc.sync.dma_start(dfm_t[:], dfm_in[:])
            nc.sync.dma_start(ident_t[:], ident_in[:])
            nc.vector.tensor_copy(ident_bf[:], ident_t[:])
            nc.sync.dma_start(h0[:], x_in[:])
            for s in range(2):
                nc.sync.dma_start(idx_res[s][:], idx_in[s][:])
                nc.sync.dma_start(ids_res[s][:], ids_in[s][:])
            nc.gpsimd.iota(iota_seg_f[:], pattern=[[0, SEGC], [1, 128]],
                           base=0, channel_multiplier=0,
                           allow_small_or_imprecise_dtypes=True)

            def bs(b):
                return slice(b * 128, (b + 1) * 128)

            def emit_g_dmas(half):
                """One contiguous DMA: g_nm cols -> chunk-major slice rows."""
                if half == 0:
                    nc.sync.dma_start(
                        ap3(g_slice[0], 0, [[HALF_A, 128], [1, HALF_A]]),
                        g_nm[:, 0:HALF_A])
                else:
                    nc.sync.dma_start(
                        ap3(g_slice[1], 0, [[HALF_B, 128], [1, HALF_B]]),
                        g_nm[:, HALF_A:NPAD])

            mp = tc.alloc_tile_pool(name="msg", bufs=MSGBUF)
            op = tc.alloc_tile_pool(name="oh", bufs=OHBUF)
            iep = tc.alloc_tile_pool(name="ie", bufs=IEBUF)
            tp = tc.alloc_tile_pool(name="meta", bufs=3)
            blp = (tc.alloc_tile_pool(name="blind", bufs=int(os.environ.get("GCN_BLBUF", 4)))
                   if (BLIND_GATHER or PREGATHER) else None)

            qctr = [0]
            PREP = os.environ.get("GCN_PREP", "") == "1"
            dma_sems = ([nc.alloc_semaphore(f"swdge_dma{q}") for q in range(NQ)]
                        if PREP else None)
            if PREGATHER:
                # all gathers bunched up-front (timing experiment): same
                # total SWDGE volume as 16 convs' worth, no interleaving
                for _cv in range(NCONV):
                    for s in range(2):
                        for si in range(len(calls[s])):
                            off, n, c0, nch = calls[s][si]
                            bt = blp.tile([128, SEGC, 128], bf16, tag="blind",
                                          name="bt")
                            q = qctr[0] % NQ
                            if PREP:
                                nc.gpsimd.dma_gather(
                                    bt[:, :nch, :], g_full[s][:],
                                    idx_res[s][:, off // 16:(off + n) // 16],
                                    num_idxs=n, num_idxs_reg=n, elem_size=128,
                                    single_packet=False, prepare_only=True,
                                    sem=dma_sems[q], queue_num=q)
                                nc.gpsimd.trigger_dma(count=None, queue_num=q)
                            else:
                                nc.gpsimd.dma_gather(
                                    bt[:, :nch, :], g_full[s][:],
                                    idx_res[s][:, off // 16:(off + n) // 16],
                                    num_idxs=n, num_idxs_reg=n, elem_size=128,
                                    single_packet=False,
                                    queue_num=q)
                            qctr[0] += 1

            for cv in range(NCONV):
                h_cur = hb[cv % 2]
                h_nxt = hb[(cv + 1) % 2]
                w_t = tp.tile([128, 128], f32, tag="wt", bufs=2)
                nc.sync.dma_start(w_t[:], w_in[:, cv * 128:(cv + 1) * 128])
                wsl = w_t[:]

                # --- phase 1: g = dis * (h @ W), node-major; fire halves ---
                for b in range(NB):
                    ph = php.tile([128, 128], f32, tag="ph")
                    nc.tensor.matmul(ph[:], h_cur[:, bs(b)], wsl,
                                     start=True, stop=True)
                    nc.scalar.activation(g_nm[:, bs(b)], ph[:], ACTF.Copy,
                                         scale=dnm_t[:, b:b + 1])
                    if b == 24:
                        if not SKIP_GDMA:
                            emit_g_dmas(0)
                        if not SKIP_COLL:
                            nc.gpsimd.collective_compute(
                                "AllGather", AT.bypass, replica_groups=rg,
                                ins=[g_slice[0][:]], outs=[g_full[0][:]])
                if not SKIP_GDMA:
                    emit_g_dmas(1)
                # NOTE: AllGather-b is emitted after pass-A's gathers are
                # enqueued so it does not block their descriptor-gen on the
                # in-order Pool queue (the collective sits on gpsimd).

                # --- phase 2: scatter-add, stream-major, selfloop folded ---
                emitted = [-1, -1]
                msg_t = [{}, {}]
                oh_t = [{}, {}]

                def emit_seg(s):
                    si = emitted[s] + 1
                    off, n, c0, nch = calls[s][si]
                    msg = mp.tile([128, SEGC, 128], bf16, tag="msg")
                    if BLIND_GATHER:
                        bt = blp.tile([128, SEGC, 128], bf16, tag="blind",
                                      name="bt")
                        nc.gpsimd.dma_gather(
                            bt[:, :nch, :], g_full[s][:],
                            idx_res[s][:, off // 16:(off + n) // 16],
                            num_idxs=n, num_idxs_reg=n, elem_size=128,
                            single_packet=False,
                            queue_num=qctr[0] % NQ)
                        qctr[0] += 1
                        nc.vector.memset(msg[:, :nch, :], 0.0)
                    elif not (SKIP_GATHER or PREGATHER):
                        nc.gpsimd.dma_gather(
                            msg[:, :nch, :], g_full[s][:],
                            idx_res[s][:, off // 16:(off + n) // 16],
                            num_idxs=n, num_idxs_reg=n, elem_size=128,
                            single_packet=False,
                            queue_num=qctr[0] % NQ)
                        qctr[0] += 1
                    else:
                        nc.vector.memset(msg[:, :nch, :], 0.0)
                    oh = op.tile([128, SEG], bf16, tag="oh")
                    sl = ids_res[s][:, c0:c0 + nch]
                    in1 = bass.AP(sl.tensor, sl.offset, sl.ap + [[0, 128]])
                    mode = OH_MODE if OH_MODE != 2 else (si % 2)
                    if mode == 3:
                        # one-hots precomputed on host, streamed via HWDGE
                        nc.sync.dma_start(
                            oh[:, :nch * 128],
                            oh_in[s][:, c0 * 128:(c0 + nch) * 128])
                    elif mode == 1:
                        ie = iep.tile([128, SEG], bf16, tag="ie")
                        nc.scalar.activation(
                            ie[:].rearrange("p (c d) -> p c d", d=128)[:, :nch, :],
                            in1, ACTF.Copy)
                        nc.vector.tensor_tensor(
                            oh[:, :nch * 128], iota_seg_f[:, :nch * 128],
                            ie[:, :nch * 128], AT.is_equal)
                    else:
                        nc.vector.tensor_tensor(
                            oh[:].rearrange("p (c d) -> p c d", d=128)[:, :nch, :],
                            iota_seg_f[:].rearrange("p (c d) -> p c d", d=128)[:, :nch, :],
                            in1, AT.is_equal)
                    msg_t[s][si] = msg
                    oh_t[s][si] = oh
                    emitted[s] = si

                def ensure(s, blk):
                    bp = min(blk, NB - 1)
                    tch = int(blk_start[s][bp]) + max(int(CPB[s][bp]), 1) - 1
                    tsi = min(tch // SEGC, len(calls[s]) - 1)
                    while emitted[s] < tsi:
                        emit_seg(s)

                for s in range(2):
                    if s == 1 and not SKIP_COLL:
                        nc.gpsimd.collective_compute(
                            "AllGather", AT.bypass, replica_groups=rg,
                            ins=[g_slice[1][:]], outs=[g_full[1][:]])
                    for b in range(NB):
                        nch_b = 0 if SKIP_STREAMS else int(CPB[s][b])
                        if s == 1 and nch_b == 0:
                            continue
                        if not SKIP_STREAMS:
                            ensure(s, b + PF)
                        pa = pap.tile([128, 128], f32, tag="pagg")
                        started = False
                        if s == 0 and not NOSELF:
                            nc.tensor.matmul(pa[:], g_nm[:, bs(b)], ident_bf[:],
                                             start=True, stop=(nch_b == 0))
                            started = True
                        c_lo = int(blk_start[s][b])
                        for j in range(nch_b):
                            ch = c_lo + j
                            si = ch // SEGC
                            jj = ch % SEGC
                            nc.tensor.matmul(
                                pa[:], msg_t[s][si][:, jj, :],
                                oh_t[s][si][:, jj * 128:(jj + 1) * 128],
                                start=(not started),
                                stop=(j == nch_b - 1))
                            started = True
                        if not started:
                            continue
                        dtgt = scr[:, 0:128] if FAKE_DRAIN else h_nxt[:, bs(b)]
                        if s == 0:
                            nc.vector.tensor_copy(dtgt, pa[:])
                        else:
                            nc.vector.tensor_tensor(dtgt, pa[:], dtgt, AT.add)

                # --- epilogue: h' = relu?(dis * agg + bias), quarter-row
                # pieces so next conv's h@W can start early ---
                QN = NPAD // 4
                for q in range(4 if not NOEPI else 0):
                    qs = slice(q * QN, (q + 1) * QN)
                    nc.vector.tensor_tensor(h_nxt[:, qs], h_nxt[:, qs],
                                            dfm_t[:, qs], AT.mult)
                    if cv % 2 == 0:
                        nc.scalar.activation(h_nxt[:, qs], h_nxt[:, qs],
                                             ACTF.Relu,
                                             bias=b_t[:, cv:cv + 1], scale=1.0)
                    else:
                        nc.vector.tensor_scalar(h_nxt[:, qs], h_nxt[:, qs],
                                                b_t[:, cv:cv + 1], None, AT.add)

            for p in ((blp,) if (BLIND_GATHER or PREGATHER) else ()) + (tp, iep, op, mp):
                p.release()

            # ---- mean pool + AllReduce ----
            h_fin = hb[NCONV % 2]
            if DUMP_H:
                nc.sync.dma_start(hdump_t[:], h_fin[:])
            tailp = tc.alloc_tile_pool(name="tail", bufs=1)
            pool_tile = tailp.tile([128, NB * NG], f32, tag="poolm")
            nc.sync.dma_start(pool_tile[:], pool_in[:])
            hnm = tailp.tile([128, NPAD], f32, tag="hnm")
            for b in range(NB):
                pt = php.tile([128, 128], f32, tag="ph")
                nc.tensor.transpose(pt[:], h_fin[:, bs(b)], ident_t[:])
                nc.vector.tensor_copy(hnm[:, bs(b)], pt[:])
            ppool = pap.tile([NG, 128], f32, tag="ppool", bufs=1)
            for b in range(NB):
                nc.tensor.matmul(ppool[:], pool_tile[:, b * NG:(b + 1) * NG],
                                 hnm[:, bs(b)], start=(b == 0),
                                 stop=(b == NB - 1))
            pres = sp.tile([NG, 128], f32, tag="pres")
            nc.vector.tensor_copy(pres[:], ppool[:])
            nc.sync.dma_start(ar_in[:], pres[:])
            if not SKIP_COLL:
                nc.gpsimd.collective_compute(
                    "AllReduce", AT.add, replica_groups=rg,
                    ins=[ar_in[:]], outs=[ar_out[:]])
            ores = sp.tile([NG, 128], f32, tag="ores")
            nc.sync.dma_start(ores[:], ar_out[:] if not SKIP_COLL else ar_in[:])
            nc.sync.dma_start(out_t[:], ores[:])
            tailp.release()
    return nc


def kernel(x, edge_index, batch, W1, b1, W2, b2, _want_trace=False, _want_res=False):
    x = np.asarray(x)
    edge_index = np.asarray(edge_index)
    batch = np.asarray(batch)
    W1, b1, W2, b2 = (np.asarray(a) for a in (W1, b1, W2, b2))

    st = host_prep(x, edge_index, batch)

    wcat = np.zeros((128, 2 * L * 128), np.float32)
    bcat = np.zeros((128, 2 * L), np.float32)
    for l in range(L):
        wcat[:, (2 * l) * 128:(2 * l + 1) * 128] = W1[l]
        wcat[:, (2 * l + 1) * 128:(2 * l + 2) * 128] = W2[l]
        bcat[:, 2 * l] = b1[l]
        bcat[:, 2 * l + 1] = b2[l]
    wcat = np.ascontiguousarray(wcat[:, :NCONV * 128])
    bcat = np.ascontiguousarray(bcat[:, :NCONV])

    nc = bacc.Bacc("TRN2", target_bir_lowering=False, debug=False,
                   enable_asserts=False, num_devices=C,
                   num_swdge_queues=NQ)
    build_program(nc, st)
    nc.compile()

    ident = np.eye(128, dtype=np.float32)
    in_maps = []
    for c in range(C):
        in_maps.append({
            "x_fm": st["x_fm"][c],
            "wcat": wcat, "bcat": bcat,
            "dis_nm": st["dis_nm"][c], "dis_fm": st["dis_fm"][c],
            "pool_t": st["pool_t"][c], "ident": ident,
            "idx0": st["idx_host"][0][c], "idx1": st["idx_host"][1][c],
            "ids0": st["ids_host"][0][c], "ids1": st["ids_host"][1][c],
            **({"ohd0": st["oh_host"][0][c], "ohd1": st["oh_host"][1][c]}
               if OH_MODE == 3 else {}),
        })

    res = run_bass_kernel_spmd(nc, in_maps, core_ids=list(range(C)),
                               trace=_want_trace)
    out = res.results[0]["out"].astype(np.float32)
    if _want_trace or _want_res:
        return out, res
    return out
